# revision 10
# baseline (speedup 1.0000x reference)
"""Cross-modal triplet loss (hardest pos/neg mining) on 8 TRN2 NeuronCores.

Strategy
--------
Rows of the four 4096x4096 distance matrices are sharded across 8 cores
(512 rows each).  On the host we sort rows/columns by target id so the
same-identity mask becomes one contiguous column range per row; each core's
column order is additionally rotated so its own diagonal block lands near
column 64, which keeps every class range inside a static 320-wide window
[128b, 128b+320) of the first PSUM half for row-block b.

Device-side, per (128-row block, matrix, 2048-col PSUM half), all in bf16
on the PE (1 cycle/col vs 4 for fp32r):

  PE   : F = 2*a_i.b_j - (sq_b[j]-128) - 896*mask(i,j), built from
         (a) a K=2 "bias" matmul (ones2 x [hi;lo] split of -(sq_b-128)),
         (b) a K=48 "mask" matmul (class-membership one-hots x -896*class
             one-hots over the 320-col window) that poisons same-class
             columns, and
         (c) the K=128 data matmuls (2a x b).
  With class columns poisoned 896 below any genuine value, the hardest
  negative is a plain row max of F and the hardest positive is a plain row
  MIN over the static 320-col window (only class cells can win it) -- no
  masked reduce needed anywhere.

  Scans are routed across the three engines that matter (32 [128,2048]
  tiles/core must be scanned):
    'dve' : DVE native reduce_max straight from PSUM (f32, ~2.3us)
    'cdve': ACT copies PSUM->SBUF bf16 (~1.8us), DVE reduces the packed
            bf16 at 2x (~1.2us)
    'cgps': ACT copies PSUM->SBUF bf16, GpSimd tree-maxes it (GpSimd has
            no PSUM port, so it can only help on SBUF copies)

  ACT  : relu(-x + sq_a+128 [-896]) and sqrt(x + 1e-12) on the [128,8]
         reduced columns.
  DVE  : margin terms relu(ap - an + 0.3) and (ap < an) counts, accumulated
         across blocks into a [128, 2] partial-sum tile.

The host sums the 8 x [128, 2] partials and divides by 6*n.  Sorting is a
permutation of rows, and loss/prec are means over rows, so no un-permutation
is needed.
"""

import sys

import numpy as np

for _p in ("/opt/trn_rl_repo", "/root/.axon_site/_ro/trn_rl_repo"):
    if _p not in sys.path:
        sys.path.append(_p)

import ml_dtypes

import concourse.bacc as bacc
import concourse.mybir as mybir
import concourse.tile as tile
from concourse.bass_utils import run_bass_kernel_spmd

N = 4096
D = 128
NCORES = 8
RPC = N // NCORES          # rows per core
NBLK = RPC // 128          # row blocks of 128 per core
HALF = 2048                # columns per PSUM tile (4 banks)
WIN = 320                  # static window containing every class range of a block
ROT_MARGIN = 64            # column rotation margin (max supported class size)
KM = 48                    # padded class-membership rows for the mask matmul
MARGIN = 0.3
MASKVAL = 896.0            # poison offset for same-class columns (bf16-exact)
SQB_SHIFT = 128.0          # center of sq_b (= D); keeps bias rows small in bf16

F32 = mybir.dt.float32
BF16 = mybir.dt.bfloat16
OP = mybir.AluOpType
ACTF = mybir.ActivationFunctionType
AX = mybir.AxisListType

BF = ml_dtypes.bfloat16

# Scan routing per (block, mat, half) tile, spread so adjacent tiles land on
# different engines.  Tuned for DVE ~= ACT busy time; GpSimd elementwise is
# ~2x slower than DVE so it gets a small share.
_MODES = (
    "cdve", "dve", "cdve", "dve", "cdve", "dve", "cdve", "dve",
    "dve", "cdve", "dve", "cdve", "dve", "cdve", "dve", "cdve",
    "cdve", "dve", "cdve", "dve", "cdve", "dve", "cdve", "dve",
    "dve", "cdve", "dve", "cdve", "dve", "cdve", "dve", "cdve",
)


def _mode(b, mi, h):
    return _MODES[(b * 8 + mi * 2 + h) % len(_MODES)]


def _build_program():
    nc = bacc.Bacc(
        "TRN2",
        target_bir_lowering=False,
        debug=False,
        num_devices=NCORES,
    )

    rhs1_d = nc.dram_tensor("rhs1", [D, N], BF16, kind="ExternalInput")
    rhs2_d = nc.dram_tensor("rhs2", [D, N], BF16, kind="ExternalInput")
    bias1_d = nc.dram_tensor("bias1", [2, N], BF16, kind="ExternalInput")
    bias2_d = nc.dram_tensor("bias2", [2, N], BF16, kind="ExternalInput")
    lhs1_d = nc.dram_tensor("lhs1", [D, RPC], BF16, kind="ExternalInput")
    lhs2_d = nc.dram_tensor("lhs2", [D, RPC], BF16, kind="ExternalInput")
    memb_d = nc.dram_tensor("memb", [KM, NBLK * 128], BF16, kind="ExternalInput")
    mrhs_d = nc.dram_tensor("mrhs", [KM, NBLK * WIN], BF16, kind="ExternalInput")
    ones2_d = nc.dram_tensor("ones2", [2, 128], BF16, kind="ExternalInput")
    pack_d = nc.dram_tensor("pack", [128, 16], F32, kind="ExternalInput")
    out_d = nc.dram_tensor("out", [128, 2], F32, kind="ExternalOutput")

    with tile.TileContext(nc) as tc:
        with (
            tc.tile_pool(name="consts", bufs=1) as cpool,
            tc.tile_pool(name="work", bufs=2) as wpool,
            tc.tile_pool(name="copies", bufs=4) as cbpool,
            tc.tile_pool(name="gpsscratch", bufs=2) as gpool,
            tc.tile_pool(name="ps", bufs=4, space="PSUM") as pspool,
        ):
            # small/first-needed DMAs first
            ones2 = cpool.tile([2, 128], BF16, tag="ones2")
            nc.sync.dma_start(out=ones2[:, :], in_=ones2_d[:, :])
            bias1 = cpool.tile([2, N], BF16, tag="bias1")
            nc.sync.dma_start(out=bias1[:, :], in_=bias1_d[:, :])
            bias2 = cpool.tile([2, N], BF16, tag="bias2")
            nc.sync.dma_start(out=bias2[:, :], in_=bias2_d[:, :])
            memb = cpool.tile([KM, NBLK * 128], BF16, tag="memb")
            nc.sync.dma_start(out=memb[:, :], in_=memb_d[:, :])
            mrhs = cpool.tile([KM, NBLK * WIN], BF16, tag="mrhs")
            nc.sync.dma_start(out=mrhs[:, :], in_=mrhs_d[:, :])
            pack = cpool.tile([128, 16], F32, tag="pack")
            nc.sync.dma_start(out=pack[:, :], in_=pack_d[:, :])
            lhs1 = cpool.tile([D, RPC], BF16, tag="lhs1")
            nc.sync.dma_start(out=lhs1[:, :], in_=lhs1_d[:, :])
            lhs2 = cpool.tile([D, RPC], BF16, tag="lhs2")
            nc.sync.dma_start(out=lhs2[:, :], in_=lhs2_d[:, :])
            # rhs halves, in consumption order
            rhs = {}
            for h in range(2):
                for side, dram in ((1, rhs1_d), (2, rhs2_d)):
                    t = cpool.tile([D, HALF], BF16, tag=f"rhs{side}h{h}")
                    nc.sync.dma_start(
                        out=t[:, :], in_=dram[:, h * HALF : (h + 1) * HALF]
                    )
                    rhs[(side, h)] = t

            zeros6 = cpool.tile([128, 6], F32, tag="zeros6")
            nc.gpsimd.memset(zeros6[:, :], 0.0)
            eps1 = cpool.tile([128, 1], F32, tag="eps1")
            nc.gpsimd.memset(eps1[:, :], 1e-12)
            accum = cpool.tile([128, 2], F32, tag="accum")
            nc.vector.memset(accum[:, :], 0.0)

            # (data lhsT, rhs side, bias tile) per distance matrix:
            # r=(m1,m1), t=(m2,m2), rt=(m1,m2), tr=(m2,m1)
            mats = [
                (lhs1, 1, bias1),
                (lhs2, 2, bias2),
                (lhs1, 2, bias2),
                (lhs2, 1, bias1),
            ]

            for b in range(NBLK):
                acc = wpool.tile([128, 8], F32, tag="acc")
                w0 = b * 128  # window start (cols of half 0)
                for mi, (lhsT, side, bias) in enumerate(mats):
                    negtmp = wpool.tile([128, 4], BF16, tag="negtmp")
                    for q in range(4):
                        c0 = q * 1024
                        rtile = rhs[(side, q // 2)]
                        r0 = (q % 2) * 1024
                        psq = pspool.tile([128, 1024], F32, tag="ps")
                        # bias matmuls (stationary: ones2)
                        for k in range(2):
                            nc.tensor.matmul(
                                out=psq[:, k * 512 : (k + 1) * 512],
                                lhsT=ones2[:, :],
                                rhs=bias[:, c0 + k * 512 : c0 + (k + 1) * 512],
                                start=True,
                                stop=False,
                            )
                        # mask matmul over the class window (only in q0),
                        # split at PSUM bank boundaries
                        if q == 0:
                            wseg = [w0]
                            nb = (w0 // 512 + 1) * 512
                            if nb < w0 + WIN:
                                wseg.append(nb)
                            wseg.append(w0 + WIN)
                            for s0, s1 in zip(wseg[:-1], wseg[1:]):
                                nc.tensor.matmul(
                                    out=psq[:, s0:s1],
                                    lhsT=memb[:, b * 128 : (b + 1) * 128],
                                    rhs=mrhs[
                                        :, b * WIN + (s0 - w0) : b * WIN + (s1 - w0)
                                    ],
                                    start=False,
                                    stop=False,
                                )
                        # data matmuls (stationary: lhsT block)
                        for k in range(2):
                            nc.tensor.matmul(
                                out=psq[:, k * 512 : (k + 1) * 512],
                                lhsT=lhsT[:, b * 128 : (b + 1) * 128],
                                rhs=rtile[:, r0 + k * 512 : r0 + (k + 1) * 512],
                                start=False,
                                stop=(k == 1),
                            )

                        # --- extrema scan for this quarter ---
                        mode = "dve" if (b + mi + q) % 2 else "cdve"
                        if mode == "dve":
                            nc.vector.tensor_reduce(
                                out=negtmp[:, q : q + 1],
                                in_=psq[:, :],
                                axis=AX.X,
                                op=OP.max,
                            )
                            if q == 0:
                                nc.vector.tensor_reduce(
                                    out=acc[:, mi : mi + 1],
                                    in_=psq[:, w0 : w0 + WIN],
                                    axis=AX.X,
                                    op=OP.min,
                                )
                        else:
                            cb = cbpool.tile([128, 1024], BF16, tag="cb")
                            nc.scalar.activation(
                                out=cb[:, :], in_=psq[:, :], func=ACTF.Copy
                            )
                            nc.vector.tensor_reduce(
                                out=negtmp[:, q : q + 1],
                                in_=cb[:, :],
                                axis=AX.X,
                                op=OP.max,
                            )
                            if q == 0:
                                nc.vector.tensor_reduce(
                                    out=acc[:, mi : mi + 1],
                                    in_=cb[:, w0 : w0 + WIN],
                                    axis=AX.X,
                                    op=OP.min,
                                )
                    # combine the four quarters' maxes
                    nc.vector.tensor_reduce(
                        out=acc[:, 4 + mi : 5 + mi],
                        in_=negtmp[:, 0:4],
                        axis=AX.X,
                        op=OP.max,
                    )

                # --- tail: ap/an for the 4 matrices live in acc cols 0-3 / 4-7
                # acc[0:4] = min F' over class window; acc[4:8] = max F' overall
                # ap^2 = -min + (sqa - 768);  an^2 = -max + (sqa + 128)
                sq = wpool.tile([128, 8], F32, tag="sq")
                # A-side of matrices [m1, m2, m1, m2] -> even cols use sq_a of
                # m1, odd cols sq_a of m2 (for both pos 0-3 and neg 4-7).
                for cols, pbase in ((slice(0, 4), 2), (slice(4, 8), 0)):
                    a3 = acc[:, cols].rearrange("p (f two) -> p f two", two=2)
                    s3 = sq[:, cols].rearrange("p (f two) -> p f two", two=2)
                    nc.scalar.activation(
                        out=s3[:, :, 0:1],
                        in_=a3[:, :, 0:1],
                        func=ACTF.Relu,
                        scale=-1.0,
                        bias=pack[:, 4 * b + pbase : 4 * b + pbase + 1],
                    )
                    nc.scalar.activation(
                        out=s3[:, :, 1:2],
                        in_=a3[:, :, 1:2],
                        func=ACTF.Relu,
                        scale=-1.0,
                        bias=pack[:, 4 * b + pbase + 1 : 4 * b + pbase + 2],
                    )
                nc.scalar.activation(
                    out=sq[:, :], in_=sq[:, :], func=ACTF.Sqrt, bias=eps1[:, :]
                )

                # margin ranking terms over the 6 (ap, an) list pairs:
                # (0,4) (1,5) (2,6) (3,7) (2,4) (3,5)
                d6 = wpool.tile([128, 6], F32, tag="d6")
                nc.vector.scalar_tensor_tensor(
                    out=d6[:, 0:4],
                    in0=sq[:, 0:4],
                    scalar=MARGIN,
                    in1=sq[:, 4:8],
                    op0=OP.add,
                    op1=OP.subtract,
                )
                nc.vector.scalar_tensor_tensor(
                    out=d6[:, 4:6],
                    in0=sq[:, 2:4],
                    scalar=MARGIN,
                    in1=sq[:, 4:6],
                    op0=OP.add,
                    op1=OP.subtract,
                )
                # native TensorTensorReduce crashes TRN2; use TensorScalarPtr
                # (scalar_tensor_tensor) whose accum_out sums the result, then
                # chain partials into `accum` with per-partition adds.
                junk = wpool.tile([128, 6], F32, tag="junk")
                fresh = wpool.tile([128, 3], F32, tag="fresh")
                nc.vector.scalar_tensor_tensor(
                    out=junk[:, 0:6],
                    in0=d6[:, 0:6],
                    scalar=0.0,
                    in1=zeros6[:, 0:6],
                    op0=OP.max,
                    op1=OP.bypass,
                    accum_out=fresh[:, 0:1],
                )
                nc.vector.scalar_tensor_tensor(
                    out=junk[:, 0:4],
                    in0=sq[:, 0:4],
                    scalar=0.0,
                    in1=sq[:, 4:8],
                    op0=OP.add,
                    op1=OP.is_lt,
                    accum_out=fresh[:, 1:2],
                )
                nc.vector.scalar_tensor_tensor(
                    out=junk[:, 0:2],
                    in0=sq[:, 2:4],
                    scalar=0.0,
                    in1=sq[:, 4:6],
                    op0=OP.add,
                    op1=OP.is_lt,
                    accum_out=fresh[:, 2:3],
                )
                nc.vector.tensor_scalar_add(
                    out=accum[:, 0:1], in0=accum[:, 0:1], scalar1=fresh[:, 0:1]
                )
                nc.vector.tensor_scalar_add(
                    out=accum[:, 1:2], in0=accum[:, 1:2], scalar1=fresh[:, 1:2]
                )
                nc.vector.tensor_scalar_add(
                    out=accum[:, 1:2], in0=accum[:, 1:2], scalar1=fresh[:, 2:3]
                )

            nc.sync.dma_start(out=out_d[:, :], in_=accum[:, :])

    nc.compile()
    return nc


def _host_prep(modal1, modal2, targets):
    """Sort/rotate/shard the inputs; returns the 8 per-core input dicts."""
    m1 = np.ascontiguousarray(np.asarray(modal1, dtype=np.float32))
    m2 = np.ascontiguousarray(np.asarray(modal2, dtype=np.float32))
    t = np.asarray(targets).astype(np.int64).ravel()
    assert m1.shape == (N, D) and m2.shape == (N, D) and t.shape == (N,)

    order = np.argsort(t, kind="stable")
    ts = t[order]
    m1s = m1[order]
    m2s = m2[order]
    sq1 = np.einsum("nd,nd->n", m1s, m1s, dtype=np.float32).astype(np.float32)
    sq2 = np.einsum("nd,nd->n", m2s, m2s, dtype=np.float32).astype(np.float32)

    change = np.r_[True, ts[1:] != ts[:-1]]
    grp_start = np.where(change)[0]
    gidx = np.cumsum(change) - 1
    starts = grp_start[gidx]                      # class start per sorted row
    grp_end = np.r_[grp_start[1:], N]
    ends = grp_end[gidx]                          # class end per sorted row
    max_cls = int((grp_end - grp_start).max())
    assert max_cls <= ROT_MARGIN, f"class size {max_cls} exceeds rotation margin"

    m1sb = m1s.astype(BF)
    m2sb = m2s.astype(BF)
    l1b = (2.0 * m1s).astype(BF)
    l2b = (2.0 * m2s).astype(BF)

    def _bias_rows(sq, cols):
        x = -(sq[cols] - np.float32(SQB_SHIFT))
        hi = x.astype(BF)
        lo = (x - hi.astype(np.float32)).astype(BF)
        return np.ascontiguousarray(np.stack([hi, lo]))

    in_maps = []
    for c in range(NCORES):
        rot = (RPC * c - ROT_MARGIN) % N
        cols = (rot + np.arange(N)) % N
        rows = np.arange(RPC * c, RPC * (c + 1))

        cs_loc = (starts[rows] - rot) % N
        ce_loc = (ends[rows] - rot) % N
        assert (cs_loc >= 1).all() and (ce_loc <= RPC + 2 * ROT_MARGIN).all()
        assert (ce_loc > cs_loc).all()
        blk = rows % RPC // 128
        ps_w = cs_loc - 128 * blk
        pe_w = ce_loc - 128 * blk
        assert (ps_w >= 0).all() and (pe_w <= WIN).all()

        gcol = gidx[cols]                          # class id per local column
        memb = np.zeros((KM, NBLK * 128), np.float32)
        mrhs = np.zeros((KM, NBLK * WIN), np.float32)
        for b in range(NBLK):
            rsl = slice(128 * b, 128 * (b + 1))
            rcls = gidx[rows[rsl]]
            cls_ids = np.unique(rcls)
            assert len(cls_ids) <= KM, f"{len(cls_ids)} classes in block"
            wcls = gcol[b * 128 : b * 128 + WIN]   # class ids of window cols
            for k, cid in enumerate(cls_ids):
                memb[k, rsl] = (rcls == cid).astype(np.float32)
                mrhs[k, b * WIN : (b + 1) * WIN] = np.where(
                    wcls == cid, -MASKVAL, 0.0
                )

        pack = np.zeros((128, 16), np.float32)
        for b in range(NBLK):
            sl = slice(128 * b, 128 * (b + 1))
            pack[:, 4 * b + 0] = sq1[rows][sl] + SQB_SHIFT            # neg m1
            pack[:, 4 * b + 1] = sq2[rows][sl] + SQB_SHIFT            # neg m2
            pack[:, 4 * b + 2] = sq1[rows][sl] + SQB_SHIFT - MASKVAL
            pack[:, 4 * b + 3] = sq2[rows][sl] + SQB_SHIFT - MASKVAL

        in_maps.append(
            {
                "rhs1": np.ascontiguousarray(m1sb[cols].T),
                "rhs2": np.ascontiguousarray(m2sb[cols].T),
                "bias1": _bias_rows(sq1, cols),
                "bias2": _bias_rows(sq2, cols),
                "lhs1": np.ascontiguousarray(l1b[rows].T),
                "lhs2": np.ascontiguousarray(l2b[rows].T),
                "memb": memb.astype(BF),
                "mrhs": mrhs.astype(BF),
                "ones2": np.ones((2, 128), BF),
                "pack": pack,
            }
        )
    return in_maps


_NC_CACHE = {}


def _get_nc():
    if "nc" not in _NC_CACHE:
        _NC_CACHE["nc"] = _build_program()
    return _NC_CACHE["nc"]


def kernel(modal1_inputs, modal2_inputs, targets, _trace=False):
    in_maps = _host_prep(modal1_inputs, modal2_inputs, targets)
    nc = _get_nc()
    res = run_bass_kernel_spmd(
        nc, in_maps, core_ids=list(range(NCORES)), trace=_trace
    )
    loss_sum = 0.0
    prec_sum = 0.0
    for r in res.results:
        loss_sum += float(r["out"][:, 0].sum(dtype=np.float64))
        prec_sum += float(r["out"][:, 1].sum(dtype=np.float64))
    denom = 6.0 * N
    out = (np.float32(loss_sum / denom), np.float32(prec_sum / denom))
    if _trace:
        return out, res
    return out
